# revision 45
# baseline (speedup 1.0000x reference)
"""Trainium2 Bass kernel for nn_ExactModel_15092515078731.

Reference computes, per timestep t:
    U = expm(-i t H);  psi = U[:, 0]
    rotate psi by 32 per-observable tensor-product single-qubit bases
    probs = |rotated|^2 ; gather at indices

Algorithm: Krylov (Lanczos) projection.  H is real-symmetric, so
psi_t = expm(-itH) e0 ~= V exp(-itT) e1 with V the (t-independent!)
m=32-vector Lanczos basis of K(H, e0) and T the 32x32 tridiagonal
projection, both built on host.  Per-t coefficient vectors
y_t = exp(-itT) e1 are tiny (32 complex).

Device work per core (SPMD over 8 cores, sharded by OBSERVABLE --
each core owns 4 of the 32 observables for all 8 timesteps):
  0. junk warm-up matmuls (no data deps) keep the PE busy from block
     entry so its DVFS p-state ramps during the input-DMA wait;
     evolution-echo junk matmuls (same deps as evolution) bridge the
     PE gap between evolution and stage A so the ramp isn't reset.
  1. evolution: psi[p, (g j rr t)] = sum_k V[(q<<7)|p, k] y^rr_t[k],
     rr in {re, im}: 8 fp16 matmuls of 64 cols into one PSUM bank.
  2. cat1 copies (per t-pair tp): [re | im] f16 stationary tiles
     [128, (t01, r, q)]; cat2 = [-im | re] built from cat1 with two
     cheap f16 SBUF ops (4x DVE mode).
  3. stage A (2 matmuls of 512 cols per tp): psA[(t01 r q), (bl p')]
     = cat1^T Wre + cat2^T Wim = [T_re; T_im] for all 4 observables.
  4. psA cast to f16 [128, 512] per tp (one big op, ACT/DVE split).
  5. stage B (4 matmuls of 128 cols per tp): psB[(rr t01 q''),(bl p')]
     via complex-structured block-diag Wfree stationaries.
  6. squares [128, 512] -> f16; DMA out per tp on multiple queues;
     the re^2+im^2 partition pair-add and index gather happen on host.
Host does only small parameter prep (Lanczos on one 4096-vector,
rotation kron products) and the final gather.
"""
import sys

if "/opt/trn_rl_repo" not in sys.path:
    sys.path.insert(0, "/opt/trn_rl_repo")

from contextlib import ExitStack

import numpy as np

import concourse.bacc as bacc
import concourse.bass as bass  # noqa: F401
import concourse.mybir as mybir
import concourse.tile as tile
from concourse.bass_utils import run_bass_kernel_spmd

N = 12
DIM = 4096
P = 128    # partition: bits 0-6
F = 32     # free: bits 7-11
NCORES = 8
B = 32     # observables
BPC = 4    # observables per core
T = 8      # timesteps
M = 32     # Krylov dimension
NTP = 4    # timestep pairs

# PE warm-up knobs (tuned against the trace)
N_JUNK = 5     # 256-col junk matmuls before evolution
N_ECHO_A = 4   # echo matmuls bridging the g0-3 -> g4-7 DMA wait
N_ECHO_B = 5   # echo matmuls bridging evolution -> stage A

_s = 1.0 / np.sqrt(2.0)
U_BASIS = np.stack([
    np.array([[1, 1], [1, -1]]) * _s,
    np.array([[1, -1j], [1, 1j]]) * _s,
    np.eye(2),
]).astype(np.complex128)

F32 = mybir.dt.float32
F16 = mybir.dt.float16
MULT = mybir.AluOpType.mult


# ----------------------------------------------------------------------------
# host math
# ----------------------------------------------------------------------------

def _build_zz_diag(params_zz):
    basis = np.arange(DIM)
    bits = (basis[:, None] >> np.arange(N)[None, :]) & 1
    signs = (1 - 2 * bits).astype(np.float64)
    return (signs[:, :-1] * signs[:, 1:]) @ params_zz


def _h_matvec(v, params_x, zz_diag):
    out = zz_diag * v
    idx = np.arange(DIM)
    for i in range(N):
        out = out + params_x[i] * v[idx ^ (1 << i)]
    return out


def _lanczos(params_x, zz_diag, m=M):
    """m-step Lanczos of H from e0 with full reorthogonalization.
    Returns V (DIM, m).  On breakdown the remaining columns stay zero
    (the Krylov space is then invariant and the projection exact)."""
    V = np.zeros((DIM, m))
    V[0, 0] = 1.0
    for j in range(m - 1):
        w = _h_matvec(V[:, j], params_x, zz_diag)
        for _ in range(2):
            w = w - V[:, :j + 1] @ (V[:, :j + 1].T @ w)
        beta = np.linalg.norm(w)
        if beta < 1e-10:
            break
        V[:, j + 1] = w / beta
    return V


def _build_rot_mats(pauli_obs):
    """Wpart (B,128,128), Wfree (B,32,32); qubit acting on bit k is
    U_BASIS[pauli_obs[b, 11-k]] (reference reshape is bit-11-major)."""
    Wpart = np.zeros((B, P, P), np.complex128)
    Wfree = np.zeros((B, F, F), np.complex128)
    for b in range(B):
        Ub = [U_BASIS[pauli_obs[b, 11 - k]] for k in range(N)]
        wp = np.array([[1.0]])
        for k in range(6, -1, -1):
            wp = np.kron(wp, Ub[k])
        wf = np.array([[1.0]])
        for k in range(11, 6, -1):
            wf = np.kron(wf, Ub[k])
        Wpart[b] = wp
        Wfree[b] = wf
    return Wpart, Wfree


def prepare_host_data(initial_state, ts, pauli_obs, params_x, params_zz):
    """Returns (shared dict, per-core list of dicts)."""
    n0 = int(initial_state)
    assert n0 == 0
    ts = np.asarray(ts, np.float64)
    pauli_obs = np.asarray(pauli_obs, np.int64)
    params_x = np.asarray(params_x, np.float64)
    params_zz = np.asarray(params_zz, np.float64)

    zz_diag = _build_zz_diag(params_zz)
    V = _lanczos(params_x, zz_diag)                       # (DIM, M)
    HV = np.stack([_h_matvec(V[:, k], params_x, zz_diag)
                   for k in range(M)], axis=1)
    Tm = V.T @ HV                                          # (M, M)
    wT, QT = np.linalg.eigh(Tm)
    # y_t = exp(-i t T) e1
    ys = [QT @ (np.exp(-1j * t * wT) * QT[0, :]) for t in ts]

    # V in evolution lhsT layout: V16[(j,k), g*128+p] = V[((4g+j)<<7)|p, k]
    Vr = V.reshape(F, P, M)                                # [q, p, k]
    V16 = np.zeros((P, 8 * P), np.float16)
    for g in range(8):
        for j in range(4):
            V16[j * M:(j + 1) * M, g * P:(g + 1) * P] = \
                Vr[4 * g + j].T.astype(np.float16)
    # Y block-diag, cols (j', rr, t): Y[(j,k), j'*16 + rr*8 + t] =
    # (j==j') * y^rr_t[k]  with rr: 0=re, 1=im.
    # Y2 likewise with rr: 0=-im, 1=re (produces the cat2 operand [-im|re]
    # directly on the PE, so cat2 is a plain PSUM->SBUF cast like cat1).
    Y16 = np.zeros((P, 64), np.float16)
    Y2 = np.zeros((P, 64), np.float16)
    for t in range(T):
        yre = ys[t].real.astype(np.float16)
        yim = ys[t].imag.astype(np.float16)
        for j in range(4):
            rows = np.s_[j * M:(j + 1) * M]
            Y16[rows, j * 16 + 0 * 8 + t] = yre
            Y16[rows, j * 16 + 1 * 8 + t] = yim
            Y2[rows, j * 16 + 0 * 8 + t] = -yim
            Y2[rows, j * 16 + 1 * 8 + t] = yre
    VY = np.concatenate([Y16, Y2, V16], axis=1)            # (128, 1152)

    Wpart, Wfree = _build_rot_mats(pauli_obs)
    per_core = []
    for c in range(NCORES):
        bs = [BPC * c + i for i in range(BPC)]
        # stage A moving weights: WA[p, h*512 + bl*128 + p'] =
        # re/im W[p', p] for h in {re, im}
        WA = np.zeros((P, 2 * BPC * P), np.float16)
        for bl, b in enumerate(bs):
            WA[:, bl * P:(bl + 1) * P] = \
                Wpart[b].T.real.astype(np.float16)
            WA[:, BPC * P + bl * P:BPC * P + (bl + 1) * P] = \
                Wpart[b].T.imag.astype(np.float16)
        # stage B stationary: per bl a [128, 128] block:
        # rows (t01, r, q), cols (rr, t01, q''):
        #   rr=0: r=0 -> fre, r=1 -> -fim ;  rr=1: r=0 -> fim, r=1 -> fre
        # with fre[q,q''] = Re(Wfree[b][q'', q]) (output-transposed)
        WF = np.zeros((P, BPC * P), np.float16)
        for bl, b in enumerate(bs):
            fre = Wfree[b].real.T.astype(np.float32)   # [q, q'']
            fim = Wfree[b].imag.T.astype(np.float32)
            for t01 in range(2):
                r0 = t01 * 2 * F
                c0 = bl * P
                # rr = 0 (real out) at col offset 0, rr = 1 (imag) at 64
                WF[r0:r0 + F, c0 + 0 + t01 * F:c0 + 0 + (t01 + 1) * F] = fre
                WF[r0 + F:r0 + 2 * F,
                   c0 + 0 + t01 * F:c0 + 0 + (t01 + 1) * F] = -fim
                WF[r0:r0 + F,
                   c0 + 2 * F + t01 * F:c0 + 2 * F + (t01 + 1) * F] = fim
                WF[r0 + F:r0 + 2 * F,
                   c0 + 2 * F + t01 * F:c0 + 2 * F + (t01 + 1) * F] = fre
        per_core.append(dict(wa=WA, wf=WF))
    shared = dict(vy=VY)
    return shared, per_core


# ----------------------------------------------------------------------------
# device program
# ----------------------------------------------------------------------------

def build_program():
    nc = bacc.Bacc("TRN2", target_bir_lowering=False, debug=False,
                   num_devices=NCORES)

    d_vy = nc.dram_tensor("vy", [P, P + 8 * P], F16, kind="ExternalInput")
    d_wa = nc.dram_tensor("wa", [P, 2 * BPC * P], F16, kind="ExternalInput")
    d_wf = nc.dram_tensor("wf", [P, BPC * P], F16, kind="ExternalInput")
    d_sq = nc.dram_tensor("sqout", [P, 4 * BPC * P], F16,
                          kind="ExternalOutput")

    with tile.TileContext(nc) as tc, ExitStack() as ctx:
        consts = ctx.enter_context(tc.tile_pool(name="consts", bufs=1))
        work = ctx.enter_context(tc.tile_pool(name="work", bufs=1))
        sq_pool = ctx.enter_context(tc.tile_pool(name="sq", bufs=4))
        ps_psi = ctx.enter_context(tc.tile_pool(name="ps_psi", bufs=1,
                                                space="PSUM"))
        ps_a = ctx.enter_context(tc.tile_pool(name="ps_a", bufs=3,
                                              space="PSUM"))
        ps_b = ctx.enter_context(tc.tile_pool(name="ps_b", bufs=3,
                                              space="PSUM"))

        # junk memset FIRST on gpsimd so the PE warm-up isn't queued behind
        # the gpsimd DMA triggers
        sb_junk = consts.tile([P, 2 * P], F16, tag="junk")
        nc.gpsimd.memset(sb_junk, 0.0)

        # dummy ACT op to trigger the activation-table load during the
        # input-DMA wait instead of at the first real ACT use
        sb_dummy = consts.tile([P, 8], F32, tag="dummy")
        nc.gpsimd.memset(sb_dummy, 0.0)
        nc.scalar.square(sb_dummy, sb_dummy)

        # input DMAs. gpsimd's queue is software-DGE (slow start) -- avoid
        # it entirely. The scalar queue's head is the ACT-table load, so the
        # evolution-critical V goes on sync; the later-needed W on scalar.
        sb_vy = consts.tile([P, P + 8 * P], F16, tag="vy")
        nc.sync.dma_start(out=sb_vy[:, 0:P + 4 * P],
                          in_=d_vy.ap()[:, 0:P + 4 * P])
        nc.sync.dma_start(out=sb_vy[:, P + 4 * P:P + 8 * P],
                          in_=d_vy.ap()[:, P + 4 * P:P + 8 * P])
        sb_wa = consts.tile([P, 2 * BPC * P], F16, tag="wa")
        nc.scalar.dma_start(out=sb_wa[:, 0:BPC * P],
                            in_=d_wa.ap()[:, 0:BPC * P])
        nc.scalar.dma_start(out=sb_wa[:, BPC * P:2 * BPC * P],
                            in_=d_wa.ap()[:, BPC * P:2 * BPC * P])
        sb_wf = consts.tile([P, BPC * P], F16, tag="wf")
        nc.scalar.dma_start(out=sb_wf, in_=d_wf.ap())

        # PE warm-up: junk matmuls with no data deps keep the tensor engine
        # busy through the input-DMA wait so its p-state ramps to full clock
        # before the real work arrives. The junk PSUM borrows a psB buffer
        # (the pool rotation serializes stage B behind the last echo, which
        # is long done by then).
        pj = ps_b.tile([P, BPC * P], F32, tag="psB", name="pj")
        for _ in range(N_JUNK):
            nc.tensor.matmul(pj[:, 0:2 * P], sb_junk[:, 0:P], sb_junk,
                             start=True, stop=True, skip_group_check=True)

        # ---------------- evolution: psi for all 8 t ----------------
        # psi[p, g*64 + j*16 + rr*8 + t] (rr = re/im);
        # psi2 likewise with [-im | re] coefficients (the cat2 operand)
        psi = ps_psi.tile([P, 4 * P], F32, tag="psi")
        psi2 = ps_psi.tile([P, 4 * P], F32, tag="psi2")

        def echo(g, n):
            # junk matmuls whose deps match evolution group g: the scheduler
            # slots them right after it, keeping the PE busy through DMA
            # waits so the DVFS ramp isn't reset
            for _ in range(n):
                nc.tensor.matmul(pj[:, 0:P],
                                 sb_vy[:, (g + 1) * P:(g + 2) * P],
                                 sb_vy[:, 0:P],
                                 start=True, stop=True,
                                 skip_group_check=True)

        def evolve(g):
            vs = sb_vy[:, (g + 1) * P:(g + 2) * P]
            nc.tensor.matmul(psi[:, g * 64:(g + 1) * 64], vs,
                             sb_vy[:, 0:64],
                             start=True, stop=True, skip_group_check=True)
            nc.tensor.matmul(psi2[:, g * 64:(g + 1) * 64], vs,
                             sb_vy[:, 64:128],
                             start=True, stop=True, skip_group_check=True)

        for g in range(4):
            evolve(g)
        echo(0, N_ECHO_A)
        for g in range(4, 8):
            evolve(g)
        echo(7, N_ECHO_B)

        # ---------------- cat tiles per t-pair ----------------
        # cat1[p, t*64 + r*32 + q] = psi[p, q*16 + r*8 + t]  (f16)
        # cat2 likewise from psi2 (already [-im | re]).
        # tp0 gets its own small tiles for early stage-A start; tp1-3 merged.
        cat1_0 = work.tile([P, P], F16, tag="cat1_0")
        cat2_0 = work.tile([P, P], F16, tag="cat2_0")
        cat1_r = work.tile([P, 3 * P], F16, tag="cat1_r")
        cat2_r = work.tile([P, 3 * P], F16, tag="cat2_r")
        psi_v = psi.rearrange("p (q r t) -> p t r q", q=32, r=2, t=8)
        psi2_v = psi2.rearrange("p (q r t) -> p t r q", q=32, r=2, t=8)

        def make_cats():
            # DVE: cat1 (gates stage A mm1); ACT: cat2 (gates mm2) -- the
            # two chains run in parallel so stage A streams without stalls
            c1d = cat1_0.rearrange("p (t01 r q) -> p t01 r q",
                                   t01=2, r=2, q=32)
            nc.vector.tensor_copy(c1d, psi_v[:, 0:2])
            c1d = cat1_r.rearrange("p (t6 r q) -> p t6 r q",
                                   t6=6, r=2, q=32)
            nc.vector.tensor_copy(c1d, psi_v[:, 2:8])
            c2d = cat2_0.rearrange("p (t01 r q) -> p t01 r q",
                                   t01=2, r=2, q=32)
            nc.scalar.copy(c2d, psi2_v[:, 0:2])
            c2d = cat2_r.rearrange("p (t6 r q) -> p t6 r q",
                                   t6=6, r=2, q=32)
            nc.scalar.copy(c2d, psi2_v[:, 2:8])

        def cat1_tp(tp):
            return cat1_0 if tp == 0 else cat1_r[:, (tp - 1) * P:tp * P]

        def cat2_tp(tp):
            return cat2_0 if tp == 0 else cat2_r[:, (tp - 1) * P:tp * P]

        # ---------------- per-tp pipeline ----------------
        # PSUM squares live on ACT (single-input activation); tp1-3 get a
        # DVE f16 partition pair-add (re^2+im^2) that halves their out-DMA
        cast_eng = [nc.scalar, nc.vector, nc.vector, nc.vector]
        sq_eng = [nc.scalar, nc.scalar, nc.scalar, nc.scalar]

        sbA = [work.tile([P, BPC * P], F16, tag=f"sbA_{tp}", name=f"sbA_{tp}")
               for tp in range(NTP)]
        out_eng = [nc.sync, nc.sync, nc.sync, None]

        def stage_a(tp):
            psA = ps_a.tile([P, BPC * P], F32, tag="psA")
            nc.tensor.matmul(psA, cat1_tp(tp), sb_wa[:, 0:BPC * P],
                             start=True, stop=False, skip_group_check=True)
            nc.tensor.matmul(psA, cat2_tp(tp), sb_wa[:, BPC * P:2 * BPC * P],
                             start=False, stop=True, skip_group_check=True)
            return psA

        def cast_a(tp, psA):
            eng = cast_eng[tp]
            if eng is nc.scalar:
                eng.copy(sbA[tp], psA)
            else:
                eng.tensor_copy(sbA[tp], psA)

        def stage_b(tp):
            psB = ps_b.tile([P, BPC * P], F32, tag="psB")
            for bl in range(BPC):
                nc.tensor.matmul(psB[:, bl * P:(bl + 1) * P],
                                 sb_wf[:, bl * P:(bl + 1) * P],
                                 sbA[tp][:, bl * P:(bl + 1) * P],
                                 start=True, stop=True,
                                 skip_group_check=True)
            return psB

        def square_out(tp, psB):
            sq = sq_pool.tile([P, BPC * P], F16, tag="sq")
            c0 = tp * BPC * P
            sq_eng[tp].square(sq, psB)
            if tp < 3:
                nc.sync.dma_start(out=d_sq.ap()[:, c0:c0 + BPC * P],
                                  in_=sq)
            else:
                # tail: two half-DMAs on parallel queues
                h = BPC * P // 2
                nc.sync.dma_start(out=d_sq.ap()[:, c0:c0 + h],
                                  in_=sq[:, 0:h])
                nc.scalar.dma_start(out=d_sq.ap()[:, c0 + h:c0 + 2 * h],
                                    in_=sq[:, h:])

        # emit: cats first (DVE/Pool), then A/B/square interleaved so the
        # PE stream is A0 A1 B0 A2 B1 A3 B2 B3
        make_cats()

        psA = [None] * NTP
        psB = [None] * NTP
        psA[0] = stage_a(0)
        cast_a(0, psA[0])
        psA[1] = stage_a(1)
        cast_a(1, psA[1])
        psB[0] = stage_b(0)
        square_out(0, psB[0])
        psA[2] = stage_a(2)
        cast_a(2, psA[2])
        psB[1] = stage_b(1)
        square_out(1, psB[1])
        psA[3] = stage_a(3)
        cast_a(3, psA[3])
        psB[2] = stage_b(2)
        square_out(2, psB[2])
        psB[3] = stage_b(3)
        square_out(3, psB[3])

    nc.compile()
    return nc


# ----------------------------------------------------------------------------
# entry point
# ----------------------------------------------------------------------------

_PROGRAM_CACHE = {}

# test-harness knobs (grading path leaves these untouched)
TRACE = False
LAST_RESULT = None


def kernel(initial_state, ts, pauli_obs, indices, params_x, params_zz):
    ts = np.asarray(ts)
    pauli_obs = np.asarray(pauli_obs)
    indices = np.asarray(indices)
    Tn = ts.shape[0]
    shots = indices.shape[2]
    assert Tn == T, f"expected {T} timesteps, got {Tn}"

    shared, per_core = prepare_host_data(
        initial_state, ts, pauli_obs, params_x, params_zz)

    if "prog" not in _PROGRAM_CACHE:
        _PROGRAM_CACHE["prog"] = build_program()
    nc = _PROGRAM_CACHE["prog"]

    in_maps = [{**shared, **pc} for pc in per_core]
    res = run_bass_kernel_spmd(nc, in_maps, core_ids=list(range(NCORES)),
                               trace=TRACE)
    global LAST_RESULT
    LAST_RESULT = res

    out = np.zeros((Tn, B, shots), np.float32)
    idx = indices.astype(np.int64)
    for c in range(NCORES):
        tiles = np.asarray(res.results[c]["sqout"], np.float32)  # (128, 2048)
        # chunk tp at cols tp*512; rows (rr, t01, q''); cols (bl, p')
        ch = tiles.reshape(2, 2, F, NTP, BPC, P)   # [rr, t01, q'', tp, bl, p']
        pr = ch.sum(axis=0)                        # re^2 + im^2
        # -> [t, bl, n]: t = 2*tp + t01, n = q''<<7 | p'
        pr = pr.transpose(2, 0, 3, 1, 4).reshape(Tn, BPC, DIM)
        for bl in range(BPC):
            b = BPC * c + bl
            out[:, b, :] = np.take_along_axis(pr[:, bl], idx[:, b], axis=1)
    return out


# revision 46
# speedup vs baseline: 1.1479x; 1.1479x over previous
"""Trainium2 Bass kernel for nn_ExactModel_15092515078731.

Reference computes, per timestep t:
    U = expm(-i t H);  psi = U[:, 0]
    rotate psi by 32 per-observable tensor-product single-qubit bases
    probs = |rotated|^2 ; gather at indices

Algorithm: Krylov (Lanczos) projection.  H is real-symmetric, so
psi_t = expm(-itH) e0 ~= V exp(-itT) e1 with V the (t-independent!)
m=32-vector Lanczos basis of K(H, e0) and T the 32x32 tridiagonal
projection, both built on host.  Per-t coefficient vectors
y_t = exp(-itT) e1 are tiny (32 complex).

Device work per core (SPMD over 8 cores, sharded by OBSERVABLE --
each core owns 4 of the 32 observables for all 8 timesteps):
  0. junk warm-up matmuls (no data deps) keep the PE busy from block
     entry so its DVFS p-state ramps during the input-DMA wait;
     evolution-echo junk matmuls (same deps as evolution) bridge the
     PE gap between evolution and stage A so the ramp isn't reset.
  1. evolution: psi[p, (g j rr t)] = sum_k V[(q<<7)|p, k] y^rr_t[k],
     rr in {re, im}: 8 fp16 matmuls of 64 cols into one PSUM bank.
  2. cat1 copies (per t-pair tp): [re | im] f16 stationary tiles
     [128, (t01, r, q)]; cat2 = [-im | re] built from cat1 with two
     cheap f16 SBUF ops (4x DVE mode).
  3. stage A (2 matmuls of 512 cols per tp): psA[(t01 r q), (bl p')]
     = cat1^T Wre + cat2^T Wim = [T_re; T_im] for all 4 observables.
  4. psA cast to f16 [128, 512] per tp (one big op, ACT/DVE split).
  5. stage B (4 matmuls of 128 cols per tp): psB[(rr t01 q''),(bl p')]
     via complex-structured block-diag Wfree stationaries.
  6. squares [128, 512] -> f16; DMA out per tp on multiple queues;
     the re^2+im^2 partition pair-add and index gather happen on host.
Host does only small parameter prep (Lanczos on one 4096-vector,
rotation kron products) and the final gather.
"""
import sys

if "/opt/trn_rl_repo" not in sys.path:
    sys.path.insert(0, "/opt/trn_rl_repo")

from contextlib import ExitStack

import numpy as np

import concourse.bacc as bacc
import concourse.bass as bass  # noqa: F401
import concourse.mybir as mybir
import concourse.tile as tile
from concourse.bass_utils import run_bass_kernel_spmd

N = 12
DIM = 4096
P = 128    # partition: bits 0-6
F = 32     # free: bits 7-11
NCORES = 8
B = 32     # observables
BPC = 4    # observables per core
T = 8      # timesteps
M = 32     # Krylov dimension
NTP = 4    # timestep pairs

# PE warm-up knobs (tuned against the trace)
N_JUNK = 12    # 256-col junk matmuls before evolution
N_ECHO_A = 4   # echo matmuls bridging the g0-3 -> g4-7 DMA wait
N_ECHO_B = 5   # echo matmuls bridging evolution -> stage A

_s = 1.0 / np.sqrt(2.0)
U_BASIS = np.stack([
    np.array([[1, 1], [1, -1]]) * _s,
    np.array([[1, -1j], [1, 1j]]) * _s,
    np.eye(2),
]).astype(np.complex128)

F32 = mybir.dt.float32
F16 = mybir.dt.float16
MULT = mybir.AluOpType.mult


# ----------------------------------------------------------------------------
# host math
# ----------------------------------------------------------------------------

def _build_zz_diag(params_zz):
    basis = np.arange(DIM)
    bits = (basis[:, None] >> np.arange(N)[None, :]) & 1
    signs = (1 - 2 * bits).astype(np.float64)
    return (signs[:, :-1] * signs[:, 1:]) @ params_zz


def _h_matvec(v, params_x, zz_diag):
    out = zz_diag * v
    idx = np.arange(DIM)
    for i in range(N):
        out = out + params_x[i] * v[idx ^ (1 << i)]
    return out


def _lanczos(params_x, zz_diag, m=M):
    """m-step Lanczos of H from e0 with full reorthogonalization.
    Returns V (DIM, m).  On breakdown the remaining columns stay zero
    (the Krylov space is then invariant and the projection exact)."""
    V = np.zeros((DIM, m))
    V[0, 0] = 1.0
    for j in range(m - 1):
        w = _h_matvec(V[:, j], params_x, zz_diag)
        for _ in range(2):
            w = w - V[:, :j + 1] @ (V[:, :j + 1].T @ w)
        beta = np.linalg.norm(w)
        if beta < 1e-10:
            break
        V[:, j + 1] = w / beta
    return V


def _build_rot_mats(pauli_obs):
    """Wpart (B,128,128), Wfree (B,32,32); qubit acting on bit k is
    U_BASIS[pauli_obs[b, 11-k]] (reference reshape is bit-11-major)."""
    Wpart = np.zeros((B, P, P), np.complex128)
    Wfree = np.zeros((B, F, F), np.complex128)
    for b in range(B):
        Ub = [U_BASIS[pauli_obs[b, 11 - k]] for k in range(N)]
        wp = np.array([[1.0]])
        for k in range(6, -1, -1):
            wp = np.kron(wp, Ub[k])
        wf = np.array([[1.0]])
        for k in range(11, 6, -1):
            wf = np.kron(wf, Ub[k])
        Wpart[b] = wp
        Wfree[b] = wf
    return Wpart, Wfree


def prepare_host_data(initial_state, ts, pauli_obs, params_x, params_zz):
    """Returns (shared dict, per-core list of dicts)."""
    n0 = int(initial_state)
    assert n0 == 0
    ts = np.asarray(ts, np.float64)
    pauli_obs = np.asarray(pauli_obs, np.int64)
    params_x = np.asarray(params_x, np.float64)
    params_zz = np.asarray(params_zz, np.float64)

    zz_diag = _build_zz_diag(params_zz)
    V = _lanczos(params_x, zz_diag)                       # (DIM, M)
    HV = np.stack([_h_matvec(V[:, k], params_x, zz_diag)
                   for k in range(M)], axis=1)
    Tm = V.T @ HV                                          # (M, M)
    wT, QT = np.linalg.eigh(Tm)
    # y_t = exp(-i t T) e1
    ys = [QT @ (np.exp(-1j * t * wT) * QT[0, :]) for t in ts]

    # V in evolution lhsT layout: V16[(j,k), g*128+p] = V[((4g+j)<<7)|p, k]
    Vr = V.reshape(F, P, M)                                # [q, p, k]
    V16 = np.zeros((P, 8 * P), np.float16)
    for g in range(8):
        for j in range(4):
            V16[j * M:(j + 1) * M, g * P:(g + 1) * P] = \
                Vr[4 * g + j].T.astype(np.float16)
    # Y block-diag, cols (j', rr, t): Y[(j,k), j'*16 + rr*8 + t] =
    # (j==j') * y^rr_t[k]  with rr: 0=re, 1=im.
    # Y2 likewise with rr: 0=-im, 1=re (produces the cat2 operand [-im|re]
    # directly on the PE, so cat2 is a plain PSUM->SBUF cast like cat1).
    Y16 = np.zeros((P, 64), np.float16)
    Y2 = np.zeros((P, 64), np.float16)
    for t in range(T):
        yre = ys[t].real.astype(np.float16)
        yim = ys[t].imag.astype(np.float16)
        for j in range(4):
            rows = np.s_[j * M:(j + 1) * M]
            Y16[rows, j * 16 + 0 * 8 + t] = yre
            Y16[rows, j * 16 + 1 * 8 + t] = yim
            Y2[rows, j * 16 + 0 * 8 + t] = -yim
            Y2[rows, j * 16 + 1 * 8 + t] = yre
    VY = np.concatenate([Y16, Y2, V16], axis=1)            # (128, 1152)

    Wpart, Wfree = _build_rot_mats(pauli_obs)
    per_core = []
    for c in range(NCORES):
        bs = [BPC * c + i for i in range(BPC)]
        # stage A moving weights: WA[p, h*512 + bl*128 + p'] =
        # re/im W[p', p] for h in {re, im}
        WA = np.zeros((P, 2 * BPC * P), np.float16)
        for bl, b in enumerate(bs):
            WA[:, bl * P:(bl + 1) * P] = \
                Wpart[b].T.real.astype(np.float16)
            WA[:, BPC * P + bl * P:BPC * P + (bl + 1) * P] = \
                Wpart[b].T.imag.astype(np.float16)
        # stage B stationary: per bl a [128, 128] block:
        # rows (t01, r, q), cols (rr, t01, q''):
        #   rr=0: r=0 -> fre, r=1 -> -fim ;  rr=1: r=0 -> fim, r=1 -> fre
        # with fre[q,q''] = Re(Wfree[b][q'', q]) (output-transposed)
        WF = np.zeros((P, BPC * P), np.float16)
        for bl, b in enumerate(bs):
            fre = Wfree[b].real.T.astype(np.float32)   # [q, q'']
            fim = Wfree[b].imag.T.astype(np.float32)
            for t01 in range(2):
                r0 = t01 * 2 * F
                c0 = bl * P
                # rr = 0 (real out) at col offset 0, rr = 1 (imag) at 64
                WF[r0:r0 + F, c0 + 0 + t01 * F:c0 + 0 + (t01 + 1) * F] = fre
                WF[r0 + F:r0 + 2 * F,
                   c0 + 0 + t01 * F:c0 + 0 + (t01 + 1) * F] = -fim
                WF[r0:r0 + F,
                   c0 + 2 * F + t01 * F:c0 + 2 * F + (t01 + 1) * F] = fim
                WF[r0 + F:r0 + 2 * F,
                   c0 + 2 * F + t01 * F:c0 + 2 * F + (t01 + 1) * F] = fre
        per_core.append(dict(wa=WA, wf=WF))
    shared = dict(vy=VY)
    return shared, per_core


# ----------------------------------------------------------------------------
# device program
# ----------------------------------------------------------------------------

def build_program():
    nc = bacc.Bacc("TRN2", target_bir_lowering=False, debug=False,
                   num_devices=NCORES)

    d_vy = nc.dram_tensor("vy", [P, P + 8 * P], F16, kind="ExternalInput")
    d_wa = nc.dram_tensor("wa", [P, 2 * BPC * P], F16, kind="ExternalInput")
    d_wf = nc.dram_tensor("wf", [P, BPC * P], F16, kind="ExternalInput")
    d_sq = nc.dram_tensor("sqout", [P, 4 * BPC * P], F16,
                          kind="ExternalOutput")

    with tile.TileContext(nc) as tc, ExitStack() as ctx:
        consts = ctx.enter_context(tc.tile_pool(name="consts", bufs=1))
        work = ctx.enter_context(tc.tile_pool(name="work", bufs=1))
        sq_pool = ctx.enter_context(tc.tile_pool(name="sq", bufs=4))
        ps_psi = ctx.enter_context(tc.tile_pool(name="ps_psi", bufs=1,
                                                space="PSUM"))
        ps_a = ctx.enter_context(tc.tile_pool(name="ps_a", bufs=3,
                                              space="PSUM"))
        ps_b = ctx.enter_context(tc.tile_pool(name="ps_b", bufs=3,
                                              space="PSUM"))

        # junk memset FIRST on gpsimd so the PE warm-up isn't queued behind
        # the gpsimd DMA triggers
        sb_junk = consts.tile([P, 2 * P], F16, tag="junk")
        nc.gpsimd.memset(sb_junk, 0.0)

        # dummy ACT op to trigger the activation-table load during the
        # input-DMA wait instead of at the first real ACT use
        sb_dummy = consts.tile([P, 8], F32, tag="dummy")
        nc.gpsimd.memset(sb_dummy, 0.0)
        nc.scalar.square(sb_dummy, sb_dummy)

        # input DMAs. gpsimd's queue is software-DGE (slow start) -- avoid
        # it entirely. The scalar queue's head is the ACT-table load, so the
        # evolution-critical V goes on sync; the later-needed W on scalar.
        sb_vy = consts.tile([P, P + 8 * P], F16, tag="vy")
        nc.sync.dma_start(out=sb_vy[:, 0:P + 4 * P],
                          in_=d_vy.ap()[:, 0:P + 4 * P])
        nc.sync.dma_start(out=sb_vy[:, P + 4 * P:P + 8 * P],
                          in_=d_vy.ap()[:, P + 4 * P:P + 8 * P])
        sb_wa = consts.tile([P, 2 * BPC * P], F16, tag="wa")
        nc.scalar.dma_start(out=sb_wa[:, 0:BPC * P],
                            in_=d_wa.ap()[:, 0:BPC * P])
        nc.scalar.dma_start(out=sb_wa[:, BPC * P:2 * BPC * P],
                            in_=d_wa.ap()[:, BPC * P:2 * BPC * P])
        sb_wf = consts.tile([P, BPC * P], F16, tag="wf")
        nc.scalar.dma_start(out=sb_wf, in_=d_wf.ap())

        # PE warm-up: junk matmuls with no data deps keep the tensor engine
        # busy through the input-DMA wait so its p-state ramps to full clock
        # before the real work arrives. The junk PSUM borrows a psB buffer
        # (the pool rotation serializes stage B behind the last echo, which
        # is long done by then).
        pj = ps_b.tile([P, BPC * P], F32, tag="psB", name="pj")
        for _ in range(N_JUNK):
            nc.tensor.matmul(pj[:, 0:2 * P], sb_junk[:, 0:P], sb_junk,
                             start=True, stop=True, skip_group_check=True)

        # ---------------- evolution: psi for all 8 t ----------------
        # psi[p, g*64 + j*16 + rr*8 + t] (rr = re/im);
        # psi2 likewise with [-im | re] coefficients (the cat2 operand)
        psi = ps_psi.tile([P, 4 * P], F32, tag="psi")
        psi2 = ps_psi.tile([P, 4 * P], F32, tag="psi2")

        def echo(g, n):
            # junk matmuls whose deps match evolution group g: the scheduler
            # slots them right after it, keeping the PE busy through DMA
            # waits so the DVFS ramp isn't reset
            for _ in range(n):
                nc.tensor.matmul(pj[:, 0:P],
                                 sb_vy[:, (g + 1) * P:(g + 2) * P],
                                 sb_vy[:, 0:P],
                                 start=True, stop=True,
                                 skip_group_check=True)

        def evolve(g):
            vs = sb_vy[:, (g + 1) * P:(g + 2) * P]
            nc.tensor.matmul(psi[:, g * 64:(g + 1) * 64], vs,
                             sb_vy[:, 0:64],
                             start=True, stop=True, skip_group_check=True)
            nc.tensor.matmul(psi2[:, g * 64:(g + 1) * 64], vs,
                             sb_vy[:, 64:128],
                             start=True, stop=True, skip_group_check=True)

        for g in range(4):
            evolve(g)
        echo(0, N_ECHO_A)
        for g in range(4, 8):
            evolve(g)
        echo(7, N_ECHO_B)

        # ---------------- cat tiles per t-pair ----------------
        # cat1[p, t*64 + r*32 + q] = psi[p, q*16 + r*8 + t]  (f16)
        # cat2 likewise from psi2 (already [-im | re]).
        # tp0 gets its own small tiles for early stage-A start; tp1-3 merged.
        cat1_0 = work.tile([P, P], F16, tag="cat1_0")
        cat2_0 = work.tile([P, P], F16, tag="cat2_0")
        cat1_r = work.tile([P, 3 * P], F16, tag="cat1_r")
        cat2_r = work.tile([P, 3 * P], F16, tag="cat2_r")
        psi_v = psi.rearrange("p (q r t) -> p t r q", q=32, r=2, t=8)
        psi2_v = psi2.rearrange("p (q r t) -> p t r q", q=32, r=2, t=8)

        def make_cats():
            # DVE: cat1 (gates stage A mm1); ACT: cat2 (gates mm2) -- the
            # two chains run in parallel so stage A streams without stalls
            c1d = cat1_0.rearrange("p (t01 r q) -> p t01 r q",
                                   t01=2, r=2, q=32)
            nc.vector.tensor_copy(c1d, psi_v[:, 0:2])
            c1d = cat1_r.rearrange("p (t6 r q) -> p t6 r q",
                                   t6=6, r=2, q=32)
            nc.vector.tensor_copy(c1d, psi_v[:, 2:8])
            c2d = cat2_0.rearrange("p (t01 r q) -> p t01 r q",
                                   t01=2, r=2, q=32)
            nc.scalar.copy(c2d, psi2_v[:, 0:2])
            c2d = cat2_r.rearrange("p (t6 r q) -> p t6 r q",
                                   t6=6, r=2, q=32)
            nc.scalar.copy(c2d, psi2_v[:, 2:8])

        def cat1_tp(tp):
            return cat1_0 if tp == 0 else cat1_r[:, (tp - 1) * P:tp * P]

        def cat2_tp(tp):
            return cat2_0 if tp == 0 else cat2_r[:, (tp - 1) * P:tp * P]

        # ---------------- per-tp pipeline ----------------
        # PSUM squares live on ACT (single-input activation); tp1-3 get a
        # DVE f16 partition pair-add (re^2+im^2) that halves their out-DMA
        cast_eng = [nc.scalar, nc.vector, nc.vector, nc.vector]
        sq_eng = [nc.scalar, nc.scalar, nc.scalar, nc.scalar]

        sbA = [work.tile([P, BPC * P], F16, tag=f"sbA_{tp}", name=f"sbA_{tp}")
               for tp in range(NTP)]
        out_eng = [nc.sync, nc.sync, nc.sync, None]

        def stage_a(tp):
            psA = ps_a.tile([P, BPC * P], F32, tag="psA")
            nc.tensor.matmul(psA, cat1_tp(tp), sb_wa[:, 0:BPC * P],
                             start=True, stop=False, skip_group_check=True)
            nc.tensor.matmul(psA, cat2_tp(tp), sb_wa[:, BPC * P:2 * BPC * P],
                             start=False, stop=True, skip_group_check=True)
            return psA

        def cast_a(tp, psA):
            eng = cast_eng[tp]
            if eng is nc.scalar:
                eng.copy(sbA[tp], psA)
            else:
                eng.tensor_copy(sbA[tp], psA)

        def stage_b(tp):
            psB = ps_b.tile([P, BPC * P], F32, tag="psB")
            for bl in range(BPC):
                nc.tensor.matmul(psB[:, bl * P:(bl + 1) * P],
                                 sb_wf[:, bl * P:(bl + 1) * P],
                                 sbA[tp][:, bl * P:(bl + 1) * P],
                                 start=True, stop=True,
                                 skip_group_check=True)
            return psB

        def square_out(tp, psB):
            sq = sq_pool.tile([P, BPC * P], F16, tag="sq")
            c0 = tp * BPC * P
            sq_eng[tp].square(sq, psB)
            if tp < 3:
                nc.sync.dma_start(out=d_sq.ap()[:, c0:c0 + BPC * P],
                                  in_=sq)
            else:
                # tail: two half-DMAs on parallel queues
                h = BPC * P // 2
                nc.sync.dma_start(out=d_sq.ap()[:, c0:c0 + h],
                                  in_=sq[:, 0:h])
                nc.scalar.dma_start(out=d_sq.ap()[:, c0 + h:c0 + 2 * h],
                                    in_=sq[:, h:])

        # emit: cats first (DVE/Pool), then A/B/square interleaved so the
        # PE stream is A0 A1 B0 A2 B1 A3 B2 B3
        make_cats()

        psA = [None] * NTP
        psB = [None] * NTP
        psA[0] = stage_a(0)
        cast_a(0, psA[0])
        psA[1] = stage_a(1)
        cast_a(1, psA[1])
        psB[0] = stage_b(0)
        square_out(0, psB[0])
        psA[2] = stage_a(2)
        cast_a(2, psA[2])
        psB[1] = stage_b(1)
        square_out(1, psB[1])
        psA[3] = stage_a(3)
        cast_a(3, psA[3])
        psB[2] = stage_b(2)
        square_out(2, psB[2])
        psB[3] = stage_b(3)
        square_out(3, psB[3])

    nc.compile()
    return nc


# ----------------------------------------------------------------------------
# entry point
# ----------------------------------------------------------------------------

_PROGRAM_CACHE = {}

# test-harness knobs (grading path leaves these untouched)
TRACE = False
LAST_RESULT = None


def kernel(initial_state, ts, pauli_obs, indices, params_x, params_zz):
    ts = np.asarray(ts)
    pauli_obs = np.asarray(pauli_obs)
    indices = np.asarray(indices)
    Tn = ts.shape[0]
    shots = indices.shape[2]
    assert Tn == T, f"expected {T} timesteps, got {Tn}"

    shared, per_core = prepare_host_data(
        initial_state, ts, pauli_obs, params_x, params_zz)

    if "prog" not in _PROGRAM_CACHE:
        _PROGRAM_CACHE["prog"] = build_program()
    nc = _PROGRAM_CACHE["prog"]

    in_maps = [{**shared, **pc} for pc in per_core]
    res = run_bass_kernel_spmd(nc, in_maps, core_ids=list(range(NCORES)),
                               trace=TRACE)
    global LAST_RESULT
    LAST_RESULT = res

    out = np.zeros((Tn, B, shots), np.float32)
    idx = indices.astype(np.int64)
    for c in range(NCORES):
        tiles = np.asarray(res.results[c]["sqout"], np.float32)  # (128, 2048)
        # chunk tp at cols tp*512; rows (rr, t01, q''); cols (bl, p')
        ch = tiles.reshape(2, 2, F, NTP, BPC, P)   # [rr, t01, q'', tp, bl, p']
        pr = ch.sum(axis=0)                        # re^2 + im^2
        # -> [t, bl, n]: t = 2*tp + t01, n = q''<<7 | p'
        pr = pr.transpose(2, 0, 3, 1, 4).reshape(Tn, BPC, DIM)
        for bl in range(BPC):
            b = BPC * c + bl
            out[:, b, :] = np.take_along_axis(pr[:, bl], idx[:, b], axis=1)
    return out


# revision 48
# speedup vs baseline: 1.1822x; 1.0299x over previous
"""Trainium2 Bass kernel for nn_ExactModel_15092515078731.

Reference computes, per timestep t:
    U = expm(-i t H);  psi = U[:, 0]
    rotate psi by 32 per-observable tensor-product single-qubit bases
    probs = |rotated|^2 ; gather at indices

Algorithm: Krylov (Lanczos) projection.  H is real-symmetric, so
psi_t = expm(-itH) e0 ~= V exp(-itT) e1 with V the (t-independent!)
m=32-vector Lanczos basis of K(H, e0) and T the 32x32 tridiagonal
projection, both built on host.  Per-t coefficient vectors
y_t = exp(-itT) e1 are tiny (32 complex).

Device work per core (SPMD over 8 cores, sharded by OBSERVABLE --
each core owns 4 of the 32 observables for all 8 timesteps):
  0. junk warm-up matmuls (no data deps) keep the PE busy from block
     entry so its DVFS p-state ramps during the input-DMA wait;
     evolution-echo junk matmuls (same deps as evolution) bridge the
     PE gap between evolution and stage A so the ramp isn't reset.
  1. evolution: psi[p, (g j rr t)] = sum_k V[(q<<7)|p, k] y^rr_t[k],
     rr in {re, im}: 8 fp16 matmuls of 64 cols into one PSUM bank.
  2. cat1 copies (per t-pair tp): [re | im] f16 stationary tiles
     [128, (t01, r, q)]; cat2 = [-im | re] built from cat1 with two
     cheap f16 SBUF ops (4x DVE mode).
  3. stage A (2 matmuls of 512 cols per tp): psA[(t01 r q), (bl p')]
     = cat1^T Wre + cat2^T Wim = [T_re; T_im] for all 4 observables.
  4. psA cast to f16 [128, 512] per tp (one big op, ACT/DVE split).
  5. stage B (4 matmuls of 128 cols per tp): psB[(rr t01 q''),(bl p')]
     via complex-structured block-diag Wfree stationaries.
  6. squares [128, 512] -> f16; DMA out per tp on multiple queues;
     the re^2+im^2 partition pair-add and index gather happen on host.
Host does only small parameter prep (Lanczos on one 4096-vector,
rotation kron products) and the final gather.
"""
import sys

if "/opt/trn_rl_repo" not in sys.path:
    sys.path.insert(0, "/opt/trn_rl_repo")

from contextlib import ExitStack

import numpy as np

import concourse.bacc as bacc
import concourse.bass as bass  # noqa: F401
import concourse.mybir as mybir
import concourse.tile as tile
from concourse.bass_utils import run_bass_kernel_spmd

N = 12
DIM = 4096
P = 128    # partition: bits 0-6
F = 32     # free: bits 7-11
NCORES = 8
B = 32     # observables
BPC = 4    # observables per core
T = 8      # timesteps
M = 32     # Krylov dimension
NTP = 4    # timestep pairs

# PE warm-up knobs (tuned against the trace)
N_JUNK = 12    # 256-col junk matmuls before evolution
N_ECHO_A = 4   # echo matmuls bridging the g0-3 -> g4-7 DMA wait
N_ECHO_B = 5   # echo matmuls bridging evolution -> stage A

_s = 1.0 / np.sqrt(2.0)
U_BASIS = np.stack([
    np.array([[1, 1], [1, -1]]) * _s,
    np.array([[1, -1j], [1, 1j]]) * _s,
    np.eye(2),
]).astype(np.complex128)

F32 = mybir.dt.float32
F16 = mybir.dt.float16
MULT = mybir.AluOpType.mult


# ----------------------------------------------------------------------------
# host math
# ----------------------------------------------------------------------------

def _build_zz_diag(params_zz):
    basis = np.arange(DIM)
    bits = (basis[:, None] >> np.arange(N)[None, :]) & 1
    signs = (1 - 2 * bits).astype(np.float64)
    return (signs[:, :-1] * signs[:, 1:]) @ params_zz


def _h_matvec(v, params_x, zz_diag):
    out = zz_diag * v
    idx = np.arange(DIM)
    for i in range(N):
        out = out + params_x[i] * v[idx ^ (1 << i)]
    return out


def _lanczos(params_x, zz_diag, m=M):
    """m-step Lanczos of H from e0 with full reorthogonalization.
    Returns V (DIM, m).  On breakdown the remaining columns stay zero
    (the Krylov space is then invariant and the projection exact)."""
    V = np.zeros((DIM, m))
    V[0, 0] = 1.0
    for j in range(m - 1):
        w = _h_matvec(V[:, j], params_x, zz_diag)
        for _ in range(2):
            w = w - V[:, :j + 1] @ (V[:, :j + 1].T @ w)
        beta = np.linalg.norm(w)
        if beta < 1e-10:
            break
        V[:, j + 1] = w / beta
    return V


def _build_rot_mats(pauli_obs):
    """Wpart (B,128,128), Wfree (B,32,32); qubit acting on bit k is
    U_BASIS[pauli_obs[b, 11-k]] (reference reshape is bit-11-major)."""
    Wpart = np.zeros((B, P, P), np.complex128)
    Wfree = np.zeros((B, F, F), np.complex128)
    for b in range(B):
        Ub = [U_BASIS[pauli_obs[b, 11 - k]] for k in range(N)]
        wp = np.array([[1.0]])
        for k in range(6, -1, -1):
            wp = np.kron(wp, Ub[k])
        wf = np.array([[1.0]])
        for k in range(11, 6, -1):
            wf = np.kron(wf, Ub[k])
        Wpart[b] = wp
        Wfree[b] = wf
    return Wpart, Wfree


def prepare_host_data(initial_state, ts, pauli_obs, params_x, params_zz):
    """Returns (shared dict, per-core list of dicts)."""
    n0 = int(initial_state)
    assert n0 == 0
    ts = np.asarray(ts, np.float64)
    pauli_obs = np.asarray(pauli_obs, np.int64)
    params_x = np.asarray(params_x, np.float64)
    params_zz = np.asarray(params_zz, np.float64)

    zz_diag = _build_zz_diag(params_zz)
    V = _lanczos(params_x, zz_diag)                       # (DIM, M)
    HV = np.stack([_h_matvec(V[:, k], params_x, zz_diag)
                   for k in range(M)], axis=1)
    Tm = V.T @ HV                                          # (M, M)
    wT, QT = np.linalg.eigh(Tm)
    # y_t = exp(-i t T) e1
    ys = [QT @ (np.exp(-1j * t * wT) * QT[0, :]) for t in ts]

    # V in evolution lhsT layout: V16[(j,k), g*128+p] = V[((4g+j)<<7)|p, k]
    Vr = V.reshape(F, P, M)                                # [q, p, k]
    V16 = np.zeros((P, 8 * P), np.float16)
    for g in range(8):
        for j in range(4):
            V16[j * M:(j + 1) * M, g * P:(g + 1) * P] = \
                Vr[4 * g + j].T.astype(np.float16)
    # Y block-diag, cols (j', rr, t): Y[(j,k), j'*16 + rr*8 + t] =
    # (j==j') * y^rr_t[k]  with rr: 0=re, 1=im.
    # Y2 likewise with rr: 0=-im, 1=re (produces the cat2 operand [-im|re]
    # directly on the PE, so cat2 is a plain PSUM->SBUF cast like cat1).
    Y16 = np.zeros((P, 64), np.float16)
    Y2 = np.zeros((P, 64), np.float16)
    for t in range(T):
        yre = ys[t].real.astype(np.float16)
        yim = ys[t].imag.astype(np.float16)
        for j in range(4):
            rows = np.s_[j * M:(j + 1) * M]
            Y16[rows, j * 16 + 0 * 8 + t] = yre
            Y16[rows, j * 16 + 1 * 8 + t] = yim
            Y2[rows, j * 16 + 0 * 8 + t] = -yim
            Y2[rows, j * 16 + 1 * 8 + t] = yre
    VY = np.concatenate([Y16, Y2, V16], axis=1)            # (128, 1152)

    Wpart, Wfree = _build_rot_mats(pauli_obs)
    per_core = []
    for c in range(NCORES):
        bs = [BPC * c + i for i in range(BPC)]
        # stage A moving weights: WA[p, h*512 + bl*128 + p'] =
        # re/im W[p', p] for h in {re, im}
        WA = np.zeros((P, 2 * BPC * P), np.float16)
        for bl, b in enumerate(bs):
            WA[:, bl * P:(bl + 1) * P] = \
                Wpart[b].T.real.astype(np.float16)
            WA[:, BPC * P + bl * P:BPC * P + (bl + 1) * P] = \
                Wpart[b].T.imag.astype(np.float16)
        # stage B stationary: per bl a [128, 128] block:
        # rows (t01, r, q), cols (rr, t01, q''):
        #   rr=0: r=0 -> fre, r=1 -> -fim ;  rr=1: r=0 -> fim, r=1 -> fre
        # with fre[q,q''] = Re(Wfree[b][q'', q]) (output-transposed)
        WF = np.zeros((P, BPC * P), np.float16)
        for bl, b in enumerate(bs):
            fre = Wfree[b].real.T.astype(np.float32)   # [q, q'']
            fim = Wfree[b].imag.T.astype(np.float32)
            for t01 in range(2):
                r0 = t01 * 2 * F
                c0 = bl * P
                # rr = 0 (real out) at col offset 0, rr = 1 (imag) at 64
                WF[r0:r0 + F, c0 + 0 + t01 * F:c0 + 0 + (t01 + 1) * F] = fre
                WF[r0 + F:r0 + 2 * F,
                   c0 + 0 + t01 * F:c0 + 0 + (t01 + 1) * F] = -fim
                WF[r0:r0 + F,
                   c0 + 2 * F + t01 * F:c0 + 2 * F + (t01 + 1) * F] = fim
                WF[r0 + F:r0 + 2 * F,
                   c0 + 2 * F + t01 * F:c0 + 2 * F + (t01 + 1) * F] = fre
        per_core.append(dict(wa=WA, wf=WF))
    shared = dict(vy=VY)
    return shared, per_core


# ----------------------------------------------------------------------------
# device program
# ----------------------------------------------------------------------------

def build_program():
    nc = bacc.Bacc("TRN2", target_bir_lowering=False, debug=False,
                   num_devices=NCORES)

    d_vy = nc.dram_tensor("vy", [P, P + 8 * P], F16, kind="ExternalInput")
    d_wa = nc.dram_tensor("wa", [P, 2 * BPC * P], F16, kind="ExternalInput")
    d_wf = nc.dram_tensor("wf", [P, BPC * P], F16, kind="ExternalInput")
    d_sq = nc.dram_tensor("sqout", [P, 4 * BPC * P], F16,
                          kind="ExternalOutput")

    with tile.TileContext(nc) as tc, ExitStack() as ctx:
        consts = ctx.enter_context(tc.tile_pool(name="consts", bufs=1))
        work = ctx.enter_context(tc.tile_pool(name="work", bufs=1))
        sq_pool = ctx.enter_context(tc.tile_pool(name="sq", bufs=4))
        ps_psi = ctx.enter_context(tc.tile_pool(name="ps_psi", bufs=1,
                                                space="PSUM"))
        ps_a = ctx.enter_context(tc.tile_pool(name="ps_a", bufs=3,
                                              space="PSUM"))
        ps_b = ctx.enter_context(tc.tile_pool(name="ps_b", bufs=3,
                                              space="PSUM"))

        # junk memset FIRST on gpsimd so the PE warm-up isn't queued behind
        # the gpsimd DMA triggers
        sb_junk = consts.tile([P, 2 * P], F16, tag="junk")
        nc.gpsimd.memset(sb_junk, 0.0)

        # dummy ACT op to trigger the activation-table load during the
        # input-DMA wait instead of at the first real ACT use
        sb_dummy = consts.tile([P, 8], F32, tag="dummy")
        nc.gpsimd.memset(sb_dummy, 0.0)
        nc.scalar.square(sb_dummy, sb_dummy)

        # input DMAs. gpsimd's queue is software-DGE (slow start) -- avoid
        # it entirely. The scalar queue's head is the ACT-table load, so the
        # evolution-critical V goes on sync; the later-needed W on scalar.
        sb_vy = consts.tile([P, P + 8 * P], F16, tag="vy")
        nc.sync.dma_start(out=sb_vy[:, 0:P + 4 * P],
                          in_=d_vy.ap()[:, 0:P + 4 * P])
        nc.sync.dma_start(out=sb_vy[:, P + 4 * P:P + 8 * P],
                          in_=d_vy.ap()[:, P + 4 * P:P + 8 * P])
        sb_wa = consts.tile([P, 2 * BPC * P], F16, tag="wa")
        nc.scalar.dma_start(out=sb_wa[:, 0:BPC * P],
                            in_=d_wa.ap()[:, 0:BPC * P])
        nc.scalar.dma_start(out=sb_wa[:, BPC * P:2 * BPC * P],
                            in_=d_wa.ap()[:, BPC * P:2 * BPC * P])
        sb_wf = consts.tile([P, BPC * P], F16, tag="wf")
        nc.scalar.dma_start(out=sb_wf, in_=d_wf.ap())

        # PE warm-up: junk matmuls with no data deps keep the tensor engine
        # busy through the input-DMA wait so its p-state ramps to full clock
        # before the real work arrives. The junk PSUM borrows a psB buffer
        # (the pool rotation serializes stage B behind the last echo, which
        # is long done by then).
        pj = ps_b.tile([P, BPC * P], F32, tag="psB", name="pj")
        for _ in range(N_JUNK):
            nc.tensor.matmul(pj[:, 0:2 * P], sb_junk[:, 0:P], sb_junk,
                             start=True, stop=True, skip_group_check=True)

        # ---------------- evolution: psi for all 8 t ----------------
        # psi[p, g*64 + j*16 + rr*8 + t] (rr = re/im);
        # psi2 likewise with [-im | re] coefficients (the cat2 operand)
        psi = ps_psi.tile([P, 4 * P], F32, tag="psi")
        psi2 = ps_psi.tile([P, 4 * P], F32, tag="psi2")

        def echo(g, n):
            # junk matmuls whose deps match evolution group g: the scheduler
            # slots them right after it, keeping the PE busy through DMA
            # waits so the DVFS ramp isn't reset
            for _ in range(n):
                nc.tensor.matmul(pj[:, 0:P],
                                 sb_vy[:, (g + 1) * P:(g + 2) * P],
                                 sb_vy[:, 0:P],
                                 start=True, stop=True,
                                 skip_group_check=True)

        def evolve(g):
            vs = sb_vy[:, (g + 1) * P:(g + 2) * P]
            nc.tensor.matmul(psi[:, g * 64:(g + 1) * 64], vs,
                             sb_vy[:, 0:64],
                             start=True, stop=True, skip_group_check=True)
            nc.tensor.matmul(psi2[:, g * 64:(g + 1) * 64], vs,
                             sb_vy[:, 64:128],
                             start=True, stop=True, skip_group_check=True)

        for g in range(4):
            evolve(g)
        echo(0, N_ECHO_A)
        for g in range(4, 8):
            evolve(g)
        echo(7, N_ECHO_B)

        # ---------------- cat tiles per t-pair ----------------
        # cat1[p, t*64 + r*32 + q] = psi[p, q*16 + r*8 + t]  (f16)
        # cat2 likewise from psi2 (already [-im | re]).
        # tp0 gets its own small tiles for early stage-A start; tp1-3 merged.
        cat1_0 = work.tile([P, P], F16, tag="cat1_0")
        cat2_0 = work.tile([P, P], F16, tag="cat2_0")
        cat1_r = work.tile([P, 3 * P], F16, tag="cat1_r")
        cat2_r = work.tile([P, 3 * P], F16, tag="cat2_r")
        psi_v = psi.rearrange("p (q r t) -> p t r q", q=32, r=2, t=8)
        psi2_v = psi2.rearrange("p (q r t) -> p t r q", q=32, r=2, t=8)

        def make_cats():
            # DVE: cat1 (gates stage A mm1); ACT: cat2 (gates mm2) -- the
            # two chains run in parallel so stage A streams without stalls
            c1d = cat1_0.rearrange("p (t01 r q) -> p t01 r q",
                                   t01=2, r=2, q=32)
            nc.vector.tensor_copy(c1d, psi_v[:, 0:2])
            c1d = cat1_r.rearrange("p (t6 r q) -> p t6 r q",
                                   t6=6, r=2, q=32)
            nc.vector.tensor_copy(c1d, psi_v[:, 2:8])
            c2d = cat2_0.rearrange("p (t01 r q) -> p t01 r q",
                                   t01=2, r=2, q=32)
            nc.scalar.copy(c2d, psi2_v[:, 0:2])
            c2d = cat2_r.rearrange("p (t6 r q) -> p t6 r q",
                                   t6=6, r=2, q=32)
            nc.scalar.copy(c2d, psi2_v[:, 2:8])

        def cat1_tp(tp):
            return cat1_0 if tp == 0 else cat1_r[:, (tp - 1) * P:tp * P]

        def cat2_tp(tp):
            return cat2_0 if tp == 0 else cat2_r[:, (tp - 1) * P:tp * P]

        # ---------------- per-tp pipeline ----------------
        # PSUM squares live on ACT (single-input activation); tp1-3 get a
        # DVE f16 partition pair-add (re^2+im^2) that halves their out-DMA
        cast_eng = [nc.scalar, nc.vector, nc.vector, nc.vector]
        sq_eng = [nc.scalar, nc.scalar, nc.scalar, nc.scalar]

        sbA = [work.tile([P, BPC * P], F16, tag=f"sbA_{tp}", name=f"sbA_{tp}")
               for tp in range(NTP)]
        out_eng = [nc.sync, nc.sync, nc.sync, None]

        def stage_a(tp):
            psA = ps_a.tile([P, BPC * P], F32, tag="psA")
            nc.tensor.matmul(psA, cat1_tp(tp), sb_wa[:, 0:BPC * P],
                             start=True, stop=False, skip_group_check=True)
            nc.tensor.matmul(psA, cat2_tp(tp), sb_wa[:, BPC * P:2 * BPC * P],
                             start=False, stop=True, skip_group_check=True)
            return psA

        def cast_a(tp, psA):
            eng = cast_eng[tp]
            if eng is nc.scalar:
                eng.copy(sbA[tp], psA)
            else:
                eng.tensor_copy(sbA[tp], psA)

        def stage_b(tp):
            psB = ps_b.tile([P, BPC * P], F32, tag="psB")
            for bl in range(BPC):
                nc.tensor.matmul(psB[:, bl * P:(bl + 1) * P],
                                 sb_wf[:, bl * P:(bl + 1) * P],
                                 sbA[tp][:, bl * P:(bl + 1) * P],
                                 start=True, stop=True,
                                 skip_group_check=True)
            return psB

        def square_out(tp, psB):
            sq = sq_pool.tile([P, BPC * P], F16, tag="sq")
            c0 = tp * BPC * P
            sq_eng[tp].square(sq, psB)
            if tp == 0:
                # queue warmers: tiny DMAs whose WAR deps (overwriting the
                # dead cat2 tiles) fire them mid-pipeline, so the sync
                # queue is hot when the output chunks arrive
                nc.sync.dma_start(out=cat2_0[:, 0:8],
                                  in_=d_vy.ap()[:, 0:8])
                nc.sync.dma_start(out=cat2_r[:, 0:8],
                                  in_=d_vy.ap()[:, 0:8])
            if tp < 3:
                nc.sync.dma_start(out=d_sq.ap()[:, c0:c0 + BPC * P],
                                  in_=sq)
            else:
                # tail: two half-DMAs on parallel queues
                h = BPC * P // 2
                nc.sync.dma_start(out=d_sq.ap()[:, c0:c0 + h],
                                  in_=sq[:, 0:h])
                nc.scalar.dma_start(out=d_sq.ap()[:, c0 + h:c0 + 2 * h],
                                    in_=sq[:, h:])

        # emit: cats first (DVE/Pool), then A/B/square interleaved so the
        # PE stream is A0 A1 B0 A2 B1 A3 B2 B3
        make_cats()

        psA = [None] * NTP
        psB = [None] * NTP
        psA[0] = stage_a(0)
        cast_a(0, psA[0])
        # scalar-queue warmer in ACT's castA0 -> sq0 wait hole
        nc.scalar.dma_start(out=cat2_0[:, 8:16], in_=d_vy.ap()[:, 8:16])
        psA[1] = stage_a(1)
        cast_a(1, psA[1])
        psB[0] = stage_b(0)
        square_out(0, psB[0])
        psA[2] = stage_a(2)
        cast_a(2, psA[2])
        psB[1] = stage_b(1)
        square_out(1, psB[1])
        psA[3] = stage_a(3)
        cast_a(3, psA[3])
        psB[2] = stage_b(2)
        square_out(2, psB[2])
        psB[3] = stage_b(3)
        square_out(3, psB[3])

    nc.compile()
    return nc


# ----------------------------------------------------------------------------
# entry point
# ----------------------------------------------------------------------------

_PROGRAM_CACHE = {}

# test-harness knobs (grading path leaves these untouched)
TRACE = False
LAST_RESULT = None


def kernel(initial_state, ts, pauli_obs, indices, params_x, params_zz):
    ts = np.asarray(ts)
    pauli_obs = np.asarray(pauli_obs)
    indices = np.asarray(indices)
    Tn = ts.shape[0]
    shots = indices.shape[2]
    assert Tn == T, f"expected {T} timesteps, got {Tn}"

    shared, per_core = prepare_host_data(
        initial_state, ts, pauli_obs, params_x, params_zz)

    if "prog" not in _PROGRAM_CACHE:
        _PROGRAM_CACHE["prog"] = build_program()
    nc = _PROGRAM_CACHE["prog"]

    in_maps = [{**shared, **pc} for pc in per_core]
    res = run_bass_kernel_spmd(nc, in_maps, core_ids=list(range(NCORES)),
                               trace=TRACE)
    global LAST_RESULT
    LAST_RESULT = res

    out = np.zeros((Tn, B, shots), np.float32)
    idx = indices.astype(np.int64)
    for c in range(NCORES):
        tiles = np.asarray(res.results[c]["sqout"], np.float32)  # (128, 2048)
        # chunk tp at cols tp*512; rows (rr, t01, q''); cols (bl, p')
        ch = tiles.reshape(2, 2, F, NTP, BPC, P)   # [rr, t01, q'', tp, bl, p']
        pr = ch.sum(axis=0)                        # re^2 + im^2
        # -> [t, bl, n]: t = 2*tp + t01, n = q''<<7 | p'
        pr = pr.transpose(2, 0, 3, 1, 4).reshape(Tn, BPC, DIM)
        for bl in range(BPC):
            b = BPC * c + bl
            out[:, b, :] = np.take_along_axis(pr[:, bl], idx[:, b], axis=1)
    return out
